# revision 17
# baseline (speedup 1.0000x reference)
"""ChebConv (complex, K+1=3 hops) Trainium2 kernel over 8 NeuronCores.

Sharding: 1D node partition on destination rows (6250 rows/core), full X
replicated; each core processes exactly the edges targeting its rows, so no
collectives are needed.

Per core the computation is reorganized as, for each 21-row "group" g and
each 128-edge block b (edges sorted by destination batch/half/group, split
by col half for int16 gather indices):

  G   = Xcat[cols[block]]                  # dma_gather, [128 edges, 512] bf16
  V   = onehot(jlocal) * C6                # [128 edges, 6*21]  (DVE, batched
                                           #  per group: 2 tensor_tensor ops)
  P_q += G[:, q*128:(q+1)*128].T @ V       # PE, PSUM [128 feat, 126] x4

Gathers are batched one dma_gather per (batch-of-6-groups, col-half) — ~100
calls/core instead of ~600 — to amortize the ~1us SWDGE fixed overhead.
P holds all four spmm partial aggregates for the group, transposed
(features on partitions).  Per batch of 6 groups, 24 more PE matmuls with
signed bf16 weight tiles contract features (pbuf staged as bf16, 4x faster
on PE than fp32) and produce the row-major outputs (real/imag) directly;
bias is added during the PSUM->SBUF copy.
"""
import sys
sys.path.insert(0, '/opt/trn_rl_repo')

import numpy as np
import ml_dtypes

N = 50000
E = 1_600_000
K1 = 3
C = 256
CORES = 8
RPC = N // CORES            # 6250 rows per core
GR = 21                     # rows per group
MCOLS = 6 * GR              # 126 one-hot columns
GPB = 6                     # groups per batch
ROWS_PB = GR * GPB          # 126
NB = -(-RPC // ROWS_PB)     # 50
NGRP = NB * GPB             # 300
REAL_GRP = -(-RPC // GR)    # 298
HALF = 32768
NQ = 4                      # SWDGE queues
import os
V_MODE = os.environ.get("V_MODE", "group")            # group | group6 | block
GATHER_MODE = os.environ.get("GATHER_MODE", "batch")  # batch | group
GATHER_SPLIT = int(os.environ.get("GATHER_SPLIT", "0"))   # max blocks/call
SCRATCH = int(os.environ.get("SCRATCH", "16384"))
# ablation: comma-set of {gather, v, blockmm, final}
ABL = set(filter(None, os.environ.get("ABL", "").split(",")))


def _bf16(x):
    return x.astype(ml_dtypes.bfloat16)


def _preprocess(rows, cols, Lr, Li, weight, bias):
    rows = np.asarray(rows).astype(np.int64)
    cols = np.asarray(cols).astype(np.int64)
    core = rows // RPC
    rloc = rows - core * RPC
    g = rloc // GR
    b = g // GPB
    gl = g - b * GPB
    jl = (rloc - g * GR).astype(np.float32)

    C6 = np.empty((E, 6), np.float32)
    C6[:, 0:3] = np.asarray(Lr).T
    C6[:, 3:6] = np.asarray(Li).T

    # order all edges by (core, batch, col-half, group-in-batch); stable so
    # layout is deterministic
    colh = (cols >= HALF).astype(np.int64)
    key = (((core * NB) + b) * 2 + colh) * GPB + gl
    order = np.argsort(key, kind="stable")
    key_s = key[order]
    nbuck = CORES * NB * 2 * GPB
    bounds = np.searchsorted(key_s, np.arange(nbuck + 1))
    cnt = (bounds[1:] - bounds[:-1]).reshape(CORES, NB, 2, GPB)

    # shared block counts per (batch, half, group) = max over cores
    nblk = -(-cnt.max(axis=0) // 128)            # [NB, 2, GPB]
    for gi in range(REAL_GRP):
        bb, ll = divmod(gi, GPB)
        if nblk[bb, :, ll].sum() == 0:
            nblk[bb, 0, ll] = 1
    tot_blk = int(nblk.sum())

    # gather-order block ids: per batch, h=0 groups 0..5 then h=1 groups 0..5
    call_nblk = nblk.sum(axis=2)                 # [NB, 2]
    woff = np.zeros((NB, 2, GPB), np.int64)      # within-call offset
    woff[:, :, 1:] = np.cumsum(nblk, axis=2)[:, :, :-1]
    batch_tot = call_nblk.sum(axis=1)            # [NB]
    gb0 = np.zeros(NB, np.int64)
    gb0[1:] = np.cumsum(batch_tot)[:-1]
    ga_id = gb0[:, None, None] + woff.copy()     # [NB, 2, GPB]
    ga_id[:, 1, :] += call_nblk[:, 0][:, None]
    ga_call0 = gb0                               # start block of (b, h=0)
    ga_call1 = gb0 + call_nblk[:, 0]             # start block of (b, h=1)

    # group-order block ids: per batch, (group 0: h0,h1), (group 1: h0,h1)..
    grp_nblk = nblk.transpose(0, 2, 1)           # [NB, GPB, 2]
    gr_flat = grp_nblk.reshape(-1, 2)            # [NGRP, 2]
    gr_tot = gr_flat.sum(axis=1)                 # blocks per group
    gr_start = np.zeros(NGRP, np.int64)
    gr_start[1:] = np.cumsum(gr_tot)[:-1]

    # per-core arrays: idx in GATHER order; c6/jl in GROUP order
    per_core = []
    cols_s = cols[order]
    C6_s = C6[order]
    jl_s = jl[order]
    for c in range(CORES):
        idx16 = np.zeros(tot_blk * 128, np.int16)
        c6t = np.zeros((128, tot_blk * 6), np.float32)
        jlf = np.zeros((128, tot_blk), np.float32)
        for bb in range(NB):
            for h in range(2):
                for ll in range(GPB):
                    nb_ = int(nblk[bb, h, ll])
                    if nb_ == 0:
                        continue
                    buck = (((c * NB) + bb) * 2 + h) * GPB + ll
                    lo, hi = int(bounds[buck]), int(bounds[buck + 1])
                    ne = hi - lo
                    ga = int(ga_id[bb, h, ll])
                    gi = bb * GPB + ll
                    gr = int(gr_start[gi]) + (int(nblk[bb, 0, ll]) if h else 0)
                    if ne:
                        sl = slice(ga * 128, ga * 128 + ne)
                        idx16[sl] = (cols_s[lo:hi] - h * HALF).astype(np.int16)
                    for k in range(nb_):
                        a, e2 = k * 128, min((k + 1) * 128, ne)
                        if a >= e2:
                            break
                        c6t[0:e2 - a, (gr + k) * 6:(gr + k) * 6 + 6] = \
                            C6_s[lo + a:lo + e2]
                        jlf[0:e2 - a, gr + k] = jl_s[lo + a:lo + e2]
        # wrap idxs: idx i lives at [i%16, i//16]; replicate to 128 partitions
        idxw = np.tile(idx16.reshape(-1, 16).T, (8, 1))  # [128, tot_blk*8]
        per_core.append(dict(
            idx=np.ascontiguousarray(idxw),
            c6=np.ascontiguousarray(_bf16(c6t)),
            jl=np.ascontiguousarray(jlf),
        ))

    # weight tiles [12][128, 256] bf16: 0..5 = +W[k][fh], 6..11 = -W[k][fh]
    weight = np.asarray(weight, np.float32)
    wt = np.empty((12, 128, C), np.float32)
    for fh in range(2):
        for k in range(K1):
            wt[fh * 3 + k] = weight[k][fh * 128:(fh + 1) * 128]
            wt[6 + fh * 3 + k] = -weight[k][fh * 128:(fh + 1) * 128]
    wsb = np.ascontiguousarray(_bf16(wt.transpose(1, 0, 2).reshape(128, 12 * C)))

    biasr = np.ascontiguousarray(np.tile(np.asarray(bias, np.float32), (128, 1)))
    # V column layout is plane-major: m = s*21 + j  ->  j = m % 21
    mdiv6 = np.ascontiguousarray(
        _bf16(np.tile((np.arange(MCOLS) % GR).astype(np.float32), (128, 1))))

    return dict(nblk=nblk, tot_blk=tot_blk, call_nblk=call_nblk, woff=woff,
                ga_call0=ga_call0, ga_call1=ga_call1, gr_start=gr_start,
                per_core=per_core, wsb=wsb, biasr=biasr, mdiv6=mdiv6)


def _final_mm_list():
    """(target, q, s, wtile): target 0=real 1=imag; q = P region; s = slot."""
    mms = []
    for tgt in range(2):
        for fh in range(2):
            for k in range(K1):
                if tgt == 0:
                    mms.append((0, fh, k, fh * 3 + k))           # +W  P_r
                    mms.append((0, 2 + fh, 3 + k, 6 + fh * 3 + k))  # -W P_i
                else:
                    mms.append((1, fh, 3 + k, fh * 3 + k))       # +W  P_r
                    mms.append((1, 2 + fh, k, fh * 3 + k))       # +W  P_i
    return mms


def _build(nc, prep, repeat=1):
    import concourse.mybir as mybir
    from concourse.tile import TileContext

    f32 = mybir.dt.float32
    bf16 = mybir.dt.bfloat16
    i16 = mybir.dt.int16
    tot_blk = prep["tot_blk"]
    nblk = prep["nblk"]
    call_nblk = prep["call_nblk"]
    woff = prep["woff"]
    ga_call = [prep["ga_call0"], prep["ga_call1"]]
    gr_start = prep["gr_start"]

    xcat = nc.dram_tensor("xcat", [N, 512], bf16, kind="ExternalInput")
    idx_d = nc.dram_tensor("idx", [128, tot_blk * 8], i16, kind="ExternalInput")
    c6_d = nc.dram_tensor("c6", [128, tot_blk * 6], bf16, kind="ExternalInput")
    jl_d = nc.dram_tensor("jl", [128, tot_blk], f32, kind="ExternalInput")
    w_d = nc.dram_tensor("wt", [128, 12 * C], bf16, kind="ExternalInput")
    bias_d = nc.dram_tensor("biasr", [128, C], f32, kind="ExternalInput")
    md_d = nc.dram_tensor("mdiv6", [128, MCOLS], bf16, kind="ExternalInput")
    or_d = nc.dram_tensor("out_r", [NB * ROWS_PB, C], f32, kind="ExternalOutput")
    oi_d = nc.dram_tensor("out_i", [NB * ROWS_PB, C], f32, kind="ExternalOutput")

    mms = _final_mm_list()

    import contextlib

    import os as _os
    GBUFS = int(_os.environ.get("GBUFS", "6"))
    with TileContext(nc) as tc:
        with tc.tile_pool(name="const", bufs=1) as cpool, \
             tc.tile_pool(name="g", bufs=GBUFS) as gpool, \
             tc.tile_pool(name="v", bufs=8) as vpool, \
             tc.tile_pool(name="pb", bufs=2) as pbpool, \
             tc.tile_pool(name="os", bufs=4) as ospool, \
             tc.tile_pool(name="ps", bufs=4, space="PSUM") as pspool, \
             tc.tile_pool(name="po", bufs=2, space="PSUM") as popool:

            idx_t = cpool.tile([128, tot_blk * 8], i16)
            c6_t = cpool.tile([128, tot_blk * 6], bf16)
            jl_t = cpool.tile([128, tot_blk], f32)
            w_t = cpool.tile([128, 12 * C], bf16)
            bias_t = cpool.tile([128, C], f32)
            md_t = cpool.tile([128, MCOLS], bf16)
            for dst, src in [(idx_t, idx_d), (c6_t, c6_d), (jl_t, jl_d),
                             (w_t, w_d), (bias_t, bias_d), (md_t, md_d)]:
                nc.sync.dma_start(dst[:], src[:])

            rep_cm = tc.For_i(0, repeat, 1) if repeat > 1 else contextlib.nullcontext()
            with rep_cm:
              qn = 0
              for bt in range(NB):
                  # batched gathers: one call (and one G tile) per chunk
                  # of up to GATHER_SPLIT blocks within each col-half
                  step = GATHER_SPLIT or 8
                  gtiles = {}        # (h, chunk) -> (tile, c0)
                  chunks = []
                  for h in range(2):
                      nbc = int(call_nblk[bt, h])
                      chunks.append([(h, ci, c0, min(step, nbc - c0))
                                     for ci, c0 in
                                     enumerate(range(0, nbc, step))])
                  ilv = []
                  for i in range(max(len(chunks[0]), len(chunks[1]))):
                      for h in range(2):
                          if i < len(chunks[h]):
                              ilv.append(chunks[h][i])
                  for h, ci, c0, cn in ilv:
                      if True:
                          bs = int(ga_call[h][bt])
                          src = xcat[:] if h == 0 else xcat[HALF:, :]
                          gt = gpool.tile([128, cn * 512], bf16, tag="g")
                          gtiles[(h, ci)] = (gt, c0)
                          if "gather" in ABL:
                              nc.sync.dma_start(
                                  gt[:].rearrange("p (b e) -> p b e", e=512),
                                  xcat[h * HALF + c0 * 128:
                                       h * HALF + (c0 + cn) * 128, :]
                                      .rearrange("(b p) e -> p b e", p=128))
                          else:
                              nidx = cn * 128
                              nc.gpsimd.dma_gather(
                                  gt[:].rearrange("p (b e) -> p b e", e=512),
                                  src,
                                  idx_t[:, (bs + c0) * 8:(bs + c0 + cn) * 8],
                                  nidx, nidx, 512,
                                  queue_num=qn,
                              )
                              qn = (qn + 1) % NQ
                  pbuf = pbpool.tile([128, GPB * 504], bf16, tag="pbuf")
                  for gll in range(GPB):
                      gi = bt * GPB + gll
                      nb0 = int(nblk[bt, 0, gll])
                      nbg = nb0 + int(nblk[bt, 1, gll])
                      if nbg == 0:
                          nc.vector.memset(
                              pbuf[:].rearrange(
                                  "p (t g j) -> p t g j", t=24, g=GPB)[
                                  :, :, gll, :], 0.0)
                          continue
                      gbs = int(gr_start[gi])
                      v_t = vpool.tile([128, nbg * MCOLS], bf16, tag="v")
                      if "v" in ABL:
                          nc.vector.memset(v_t[:], 0.0)
                      elif V_MODE == "group":
                          # batched V build: V = (mdiv6 == jl) * c6, per group
                          v3 = v_t[:].rearrange("p (b m) -> p b m", m=MCOLS)
                          nc.vector.tensor_tensor(
                              v3,
                              md_t[:].unsqueeze(1)
                                  .broadcast_to((128, nbg, MCOLS)),
                              jl_t[:, gbs:gbs + nbg].unsqueeze(2)
                                  .broadcast_to((128, nbg, MCOLS)),
                              mybir.AluOpType.is_equal)
                          v4 = v_t[:].rearrange("p (b s j) -> p b s j",
                                                s=6, j=GR)
                          nc.vector.tensor_tensor(
                              v4, v4,
                              c6_t[:, gbs * 6:(gbs + nbg) * 6]
                                  .rearrange("p (b s) -> p b s", s=6)
                                  .unsqueeze(3)
                                  .broadcast_to((128, nbg, 6, GR)),
                              mybir.AluOpType.mult)
                      elif V_MODE == "group6":
                          v3 = v_t[:].rearrange("p (b m) -> p b m", m=MCOLS)
                          nc.vector.tensor_tensor(
                              v3,
                              md_t[:].unsqueeze(1)
                                  .broadcast_to((128, nbg, MCOLS)),
                              jl_t[:, gbs:gbs + nbg].unsqueeze(2)
                                  .broadcast_to((128, nbg, MCOLS)),
                              mybir.AluOpType.is_equal)
                          v4 = v_t[:].rearrange("p (b s j) -> p b s j",
                                                s=6, j=GR)
                          c63 = c6_t[:, gbs * 6:(gbs + nbg) * 6] \
                              .rearrange("p (b s) -> p b s", s=6)
                          for s in range(6):
                              nc.vector.tensor_tensor(
                                  v4[:, :, s, :], v4[:, :, s, :],
                                  c63[:, :, s:s + 1]
                                      .broadcast_to((128, nbg, GR)),
                                  mybir.AluOpType.mult)
                      else:  # per-block (baseline style)
                          for j in range(nbg):
                              gb = gbs + j
                              vj = v_t[:, j * MCOLS:(j + 1) * MCOLS]
                              nc.vector.tensor_scalar(
                                  vj, md_t[:], jl_t[:, gb:gb + 1], None,
                                  mybir.AluOpType.is_equal)
                              c6rep = c6_t[:, gb * 6:gb * 6 + 6] \
                                  .unsqueeze(2).broadcast_to((128, 6, GR))
                              nc.vector.tensor_tensor(
                                  vj.rearrange("p (s x) -> p s x", x=GR),
                                  vj.rearrange("p (s x) -> p s x", x=GR),
                                  c6rep, mybir.AluOpType.mult)
                      p_t = pspool.tile([128, 504], f32, tag="p")
                      for q in range(4):
                          for j in range(nbg if "blockmm" not in ABL else 1):
                              h = 0 if j < nb0 else 1
                              pos = int(woff[bt, h, gll]) + (j if h == 0
                                                             else j - nb0)
                              gt_, c0_ = gtiles[(h, pos // step)]
                              gcol = (pos - c0_) * 512 + q * 128
                              nc.tensor.matmul(
                                  p_t[:, q * MCOLS:(q + 1) * MCOLS],
                                  gt_[:, gcol:gcol + 128],
                                  v_t[:, j * MCOLS:(j + 1) * MCOLS],
                                  start=(j == 0),
                                  stop=(j == (nbg if "blockmm" not in ABL
                                              else 1) - 1))
                      # pbuf is plane-major over the whole batch: column
                      # t*126 + 21*gl + j with t = q*6+s; p_t cols q*126+s*21+j
                      pb_dst = pbuf[:].rearrange(
                          "p (t g j) -> p t g j", t=24, g=GPB)[:, :, gll, :]
                      nc.scalar.copy(
                          pb_dst,
                          p_t[:].rearrange("p (q s j) -> p (q s) j", q=4, s=6))
                  # final matmuls for this batch (all-bf16 operands)
                  po_r = popool.tile([128, C], f32, tag="por")
                  po_i = popool.tile([128, C], f32, tag="poi")
                  nmm = {0: 0, 1: 0}
                  for tgt, q, s, wi in (mms if "final" not in ABL
                                        else [mms[0], mms[12]]):
                      po = po_r if tgt == 0 else po_i
                      plane = q * 6 + s
                      lhsT = pbuf[:, plane * MCOLS:(plane + 1) * MCOLS]
                      nc.tensor.matmul(
                          po[:MCOLS, :], lhsT, w_t[:, wi * C:(wi + 1) * C],
                          start=(nmm[tgt] == 0),
                          stop=(nmm[tgt] == (11 if "final" not in ABL else 0)))
                      nmm[tgt] += 1
                  o_r = ospool.tile([128, C], f32, tag="or")
                  o_i = ospool.tile([128, C], f32, tag="oi")
                  nc.vector.tensor_tensor(o_r[:MCOLS, :], po_r[:MCOLS, :],
                                          bias_t[:MCOLS, :], mybir.AluOpType.add)
                  nc.vector.tensor_tensor(o_i[:MCOLS, :], po_i[:MCOLS, :],
                                          bias_t[:MCOLS, :], mybir.AluOpType.add)
                  nc.sync.dma_start(or_d[bt * ROWS_PB:(bt + 1) * ROWS_PB, :],
                                    o_r[:MCOLS, :])
                  nc.sync.dma_start(oi_d[bt * ROWS_PB:(bt + 1) * ROWS_PB, :],
                                    o_i[:MCOLS, :])


def _make_nc(prep, repeat=1):
    import concourse.bacc as bacc
    nc = bacc.Bacc("TRN2", target_bir_lowering=False, debug=False,
                   num_swdge_queues=NQ, dynamic_dma_scratch_size=SCRATCH)
    _build(nc, prep, repeat=repeat)
    nc.compile()
    return nc


def kernel(X_real, X_imag, L_real_vals, L_imag_vals, weight, bias, rows, cols):
    from concourse.bass_utils import run_bass_kernel_spmd

    prep = _preprocess(rows, cols, L_real_vals, L_imag_vals, weight, bias)
    nc = _make_nc(prep)

    xcat = _bf16(np.concatenate(
        [np.asarray(X_real, np.float32), np.asarray(X_imag, np.float32)], axis=1))
    in_maps = []
    for c in range(CORES):
        pc = prep["per_core"][c]
        in_maps.append({
            "xcat": xcat, "idx": pc["idx"], "c6": pc["c6"], "jl": pc["jl"],
            "wt": prep["wsb"], "biasr": prep["biasr"], "mdiv6": prep["mdiv6"],
        })
    res = run_bass_kernel_spmd(nc, in_maps, core_ids=list(range(CORES)))
    out_r = np.concatenate([res.results[c]["out_r"][:RPC] for c in range(CORES)], 0)
    out_i = np.concatenate([res.results[c]["out_i"][:RPC] for c in range(CORES)], 0)
    return out_r, out_i
